# revision 46
# baseline (speedup 1.0000x reference)
"""PointNet-style encoder (conv1x1 stack + ragged segment-max) on 8 Trainium2 cores.

Strategy
--------
* BN folded into the conv weights host-side; every layer becomes matmul+bias+ReLU.
* Feature-major on device: activations live as [C, points] tiles, points stream
  through the PE as the matmul free dimension in 512-point macro-tiles.
* Dense point packing: segments are NOT padded to 512-point multiples. Each
  core's ceil(B/8)=4 segments are concatenated with only <=127-point alignment
  pads (duplicated own points; exact under max-pooling). Segment boundaries are
  confined to a few shared static WINDOW tiles; window tiles are processed at
  128-column quarter granularity (static program structure), and which quarter
  belongs to which segment is per-core mask DATA. All other tiles are single
  whole pieces.
* L1 (K=3) is row-packed: two x tiles share one [35, PT] SBUF tile at partition
  strips 0-2 / 32-34, halving the x DMA trigger count per tile pair.
* f2 is stored SHIFTED: f2' = max(ps2, -b2) = relu(ps2+b2) - b2. Channel-half 1
  is one DVE tensor_scalar pass that emits both the fp16 f2' tile and the
  per-piece max (accum_out); channel-half 0 (when b2 == 0, always true for this
  reference's identity BN) is an ACT relu store plus a light DVE reduce, which
  halves the DVE cost that paces the pipeline fill. The constant shift is
  folded into b3 host-side (b3 += W3a^T b2 + W3b^T b2); the g-slot selection
  uses an additive -inf mask since shifted maxes can be negative.
* No warmup/dummy matmuls: scheduler-visible fake work was measured to poison
  the Tile scheduler's priority order (coalesced semaphore waits blocked real
  work behind it). The clock gate opens during the fill's own matmul stream.
* PSUM: psA bufs=4 (the fill was dependency-latency-bound at 2), psB3 2,
  psB4 2x single-bank per-m L4 tiles. All DMAs ride sync (x) / gpsimd (consts,
  weights) queues; the ACT queue carries no DMA triggers so h1 is never queued
  behind descriptor generation. g/U emission is tc.high_priority().
* The mid-network segment-max feeds back via the concat identity
  concat(f2, g) @ W3 = f2 @ W3[:256] + g @ W3[256:]; the g-contribution plus
  b3 becomes a per-piece bias column (table U), applied by the activation
  instruction's per-partition bias operand.
* Phase A (layers 1-2) and phase B (layers 3-4) are emitted as ONE interleaved
  pipeline: host-computed static bounds let per-slot g reductions and per-slot
  U-table batches run mid-stream, so phase A's DVE/ACT work hides under phase
  B's PE-bound matmuls.
* Per-piece layer-4 maxes are returned raw (streamed out mid-kernel in three
  DMA chunks); the host applies relu(. + b4) and the per-segment max over
  pieces (monotonicity makes this exact).
* Matmuls run in float16 (fp32 PSUM accumulate): 1 cycle/column, ~1e-3 rel err.
"""

import numpy as np
from itertools import permutations

import concourse.bass as bass
import concourse.mybir as mybir
import concourse.tile as tile
from concourse import bacc
from concourse.bass_utils import run_bass_kernel_spmd

N_CORES = 8
PT = 512  # points per macro-tile
QT = 64  # piece-alignment width inside window tiles
EPS = 1e-3  # keras BatchNormalization default epsilon

F32 = mybir.dt.float32
F16 = mybir.dt.float16
AF = mybir.ActivationFunctionType
AXX = mybir.AxisListType.X
ALU_MAX = mybir.AluOpType.max

_PROGRAM_CACHE: dict = {}


def _layout(T_cols: int, wtiles: tuple):
    """Static piece layout: window tiles split into 128-col quarters."""
    T = (T_cols + PT - 1) // PT
    lens = [min(PT, T_cols - t * PT) for t in range(T)]
    pieces = []  # (tile, a, b)
    tile_pieces = [[] for _ in range(T)]
    for t in range(T):
        if t in wtiles:
            qs = [(t, a, min(a + QT, lens[t])) for a in range(0, lens[t], QT)]
        else:
            qs = [(t, 0, lens[t])]
        for q in qs:
            tile_pieces[t].append(len(pieces))
            pieces.append(q)
    return T, lens, pieces, tile_pieces


def _build_program(T_cols: int, wtiles: tuple, S: int, bounds: tuple, smax: tuple,
                   zsh: bool = False):
    """One SPMD program for all cores.

    bounds[s]: phase-A tile count after which slot s is complete on every core.
    smax[p]: max slot id piece p has on any core. Non-decreasing in p, so the U
    table is produced in S contiguous per-slot batches instead of small chunks.
    """
    T, lens, pieces, tile_pieces = _layout(T_cols, wtiles)
    NP = len(pieces)
    NPp = NP + (NP % 2)
    # piece-column counts completed after `bounds[s]` phase-A tiles
    pbounds = [max(tile_pieces[bounds[s] - 1]) + 1 for s in range(S)]
    # U batch s covers pieces with smax == s (contiguous ranges)
    ubatch = [(min((p for p in range(NP) if smax[p] == s), default=0),
               max((p + 1 for p in range(NP) if smax[p] == s), default=0))
              for s in range(S)]
    ubatch_of_tile = [smax[max(tile_pieces[t])] for t in range(T)]

    nc = bacc.Bacc("TRN2")

    xT = nc.dram_tensor("xT", [3, T_cols], F16, kind="ExternalInput")
    mask = nc.dram_tensor("mask", [128, S, NPp], F32, kind="ExternalInput")
    maskA = nc.dram_tensor("maskA", [128, S, NPp], F32, kind="ExternalInput")
    w1 = nc.dram_tensor("w1", [35, 128], F16, kind="ExternalInput")
    w2 = nc.dram_tensor("w2", [128, 2, 128], F16, kind="ExternalInput")
    w3a = nc.dram_tensor("w3a", [128, 2, 4, 128], F16, kind="ExternalInput")
    w3b = nc.dram_tensor("w3b", [128, 2, 4, 128], F16, kind="ExternalInput")
    w4 = nc.dram_tensor("w4", [128, 4, 8, 128], F16, kind="ExternalInput")
    b1 = nc.dram_tensor("b1", [128, 1], F32, kind="ExternalInput")
    nb2 = nc.dram_tensor("nb2", [128, 2], F32, kind="ExternalInput")
    b3 = nc.dram_tensor("b3", [128, 4], F32, kind="ExternalInput")
    mx4 = nc.dram_tensor("mx4", [128, 8, NP], F32, kind="ExternalOutput")

    with tile.TileContext(nc) as tc:
        with (
            tc.tile_pool(name="const", bufs=1) as constp,
            tc.tile_pool(name="xp", bufs=6) as xp,
            tc.tile_pool(name="h1p", bufs=3) as h1p,
            tc.tile_pool(name="h3p", bufs=4) as h3p,
            tc.tile_pool(name="tmpp", bufs=4) as tmpp,
            tc.tile_pool(name="psA", bufs=4, space="PSUM") as psA,
            tc.tile_pool(name="psB3", bufs=2, space="PSUM") as psB3,
            tc.tile_pool(name="psB4", bufs=2, space="PSUM") as psB4,
        ):
            NQ = PT // QT

            # ---- constants / big buffers ----
            w1_sb = constp.tile([35, 128], F16)
            b1_sb = constp.tile([128, 1], F32)
            nb2_sb = constp.tile([128, 2], F32)
            b3_sb = constp.tile([128, 4], F32)
            w2_sb = constp.tile([128, 2, 128], F16)
            mask_sb = constp.tile([128, S, NPp], F32)
            maskA_sb = constp.tile([128, S, NPp], F32)
            w3a_sb = constp.tile([128, 2, 4, 128], F16)
            w3b_sb = constp.tile([128, 2, 4, 128], F16)
            w4_sb = constp.tile([128, 4, 8, 128], F16)

            f2_all = constp.tile([128, T, 2, NQ, QT], F16)
            Mx2_sb = constp.tile([128, 2, NP], F32)
            g_sb = constp.tile([128, 2, S], F32)
            G2_sb = constp.tile([128, 2, NPp], F16)
            U_sb = constp.tile([128, 4, NPp], F32)
            Mx4_sb = constp.tile([128, 8, NP], F32)

            warm_src = constp.tile([128, PT], F16, name="warm_src")
            warm_act = constp.tile([128, 8], F32, name="warm_act")

            xTr = xT.ap()

            # ---- prologue: the scalar (ACT) queue carries NO DMA triggers so
            # the first h1 relus aren't queued behind descriptor generation;
            # sync carries the x tiles, gpsimd everything else in need-order.
            # sync queue: first x tiles
            def dma_x_pair(ta, tb):
                """One [35, PT] tile holding x[ta] at rows 0-2 and x[tb] at 32-34."""
                xsb = xp.tile([35, PT], F16, tag="x", name=f"x_{ta}_{tb}")
                nc.sync.dma_start(out=xsb[0:3, : lens[ta]], in_=xTr[:, ta * PT : ta * PT + lens[ta]])
                if tb is not None:
                    nc.sync.dma_start(out=xsb[32:35, : lens[tb]], in_=xTr[:, tb * PT : tb * PT + lens[tb]])
                return xsb

            pre_pairs = {}
            for ta in range(0, min(10, T), 2):
                tb = ta + 1 if ta + 1 < T else None
                pre_pairs[ta] = dma_x_pair(ta, tb)

            # gpsimd queue: small consts in need-order, then big phase-B weights
            nc.gpsimd.dma_start(out=w1_sb, in_=w1.ap())
            nc.gpsimd.dma_start(out=b1_sb, in_=b1.ap())
            nc.gpsimd.dma_start(out=w2_sb, in_=w2.ap())
            nc.gpsimd.dma_start(out=nb2_sb, in_=nb2.ap())
            nc.gpsimd.dma_start(out=b3_sb, in_=b3.ap())
            nc.gpsimd.dma_start(out=w3a_sb, in_=w3a.ap())
            nc.gpsimd.dma_start(out=mask_sb, in_=mask.ap())
            nc.gpsimd.dma_start(out=maskA_sb, in_=maskA.ap())
            nc.gpsimd.dma_start(out=w3b_sb, in_=w3b.ap())
            nc.gpsimd.dma_start(out=w4_sb, in_=w4.ap())

            # touch the scalar engine early so its ACT_TABLE_LOAD happens
            # during the DMA prologue instead of gating the first real Relu
            nc.vector.memset(warm_src, 0.01)
            nc.scalar.activation(out=warm_act, in_=warm_src[:, 0:8], func=AF.Relu)

            # ---------------- emission helpers ----------------
            def emit_f2_store(t, c, ps2):
                """Shifted store f2' = max(ps2, -b2[c]) + per-piece max.

                c=1 (and c=0 when b2 != 0): one DVE tensor_scalar pass whose
                accum_out emits the piece max for free. c=0 with b2 == 0: the
                store runs on ACT (relu) and the piece max on GpSimd, keeping
                the DVE free for phase-B reduces — engines that are otherwise
                idle during the pipeline fill.
                """
                L = lens[t]
                nq = L // QT
                ps = tile_pieces[t]
                # slot-boundary tiles gate g(s): keep their c0 max fully fused
                # on the DVE (one hop) instead of ACT-store + DVE-reduce (two)
                if zsh and c == 0 and (t + 1) not in bounds:
                    nc.scalar.activation(
                        out=f2_all[:, t, 0, :nq, :], in_=ps2[:, :L], func=AF.Relu,
                        bias=nb2_sb[:, 0:1],
                    )
                    if len(ps) == 1:
                        nc.vector.tensor_reduce(
                            out=Mx2_sb[:, 0, ps[0] : ps[0] + 1],
                            in_=f2_all[:, t, 0, :nq, :],
                            axis=mybir.AxisListType.XY, op=ALU_MAX,
                        )
                    else:
                        nc.vector.tensor_reduce(
                            out=Mx2_sb[:, 0, ps[0] : ps[0] + nq],
                            in_=f2_all[:, t, 0, :nq, :],
                            axis=AXX, op=ALU_MAX,
                        )
                    return
                for p in ps:
                    _, a, b = pieces[p]
                    qa, qb = a // QT, (b + QT - 1) // QT
                    nc.vector.tensor_scalar(
                        f2_all[:, t, c, qa:qb, :],
                        ps2[:, a:b],
                        nb2_sb[:, c : c + 1],
                        None,
                        ALU_MAX,
                        ALU_MAX,
                        accum_out=Mx2_sb[:, c, p : p + 1],
                    )

            def emit_A_pair(ta, tb):
                """L1+L2 for tiles ta, tb (tb may be None); L1s row-packed."""
                if ta in pre_pairs:
                    x_sb = pre_pairs.pop(ta)
                else:
                    x_sb = dma_x_pair(ta, tb)
                ps1 = {}
                ps1[ta] = psA.tile([128, PT], F32, tag="psa", name=f"ps1_{ta}")
                nc.tensor.matmul(
                    ps1[ta][:, : lens[ta]], w1_sb[0:3, :], x_sb[0:3, : lens[ta]],
                    start=True, stop=True,
                )
                if tb is not None:
                    ps1[tb] = psA.tile([128, PT], F32, tag="psa", name=f"ps1_{tb}")
                    nc.tensor.matmul(
                        ps1[tb][:, : lens[tb]], w1_sb[32:35, :], x_sb[32:35, : lens[tb]],
                        start=True, stop=True,
                    )
                ts_list = [ta] if tb is None else [ta, tb]
                h1_sbs = {}
                for t in ts_list:
                    L = lens[t]
                    h1_sb = h1p.tile([128, PT], F16, tag="h1", name=f"h1_{t}")
                    nc.scalar.activation(out=h1_sb[:, :L], in_=ps1[t][:, :L], func=AF.Relu, bias=b1_sb[:, 0:1])
                    h1_sbs[t] = h1_sb
                for t in ts_list:
                    L = lens[t]
                    for c in range(2):
                        ps2 = psA.tile([128, PT], F32, tag="psa", name=f"ps2_{t}_{c}")
                        nc.tensor.matmul(ps2[:, :L], w2_sb[:, c, :], h1_sbs[t][:, :L], start=True, stop=True)
                        emit_f2_store(t, c, ps2)

            def emit_g(s):
                """Per-slot max over the (host-bounded) range of Mx2 columns.

                High priority: g gates the U table which gates all of phase B;
                the scheduler must not queue it behind later f2 stores.
                """
                pb = pbounds[s]
                with tc.high_priority():
                    for c in range(2):
                        tmp = tmpp.tile([128, NPp], F32, tag="tmp", name=f"tmpg_{c}_{s}")
                        nc.vector.tensor_add(tmp[:, :pb], Mx2_sb[:, c, :pb], maskA_sb[:, s, :pb])
                        nc.vector.tensor_reduce(
                            out=g_sb[:, c, s : s + 1], in_=tmp[:, :pb], axis=AXX, op=ALU_MAX
                        )

            def emit_Ubatch(k):
                """U for all pieces whose (max) slot is k, in one wide matmul."""
                c0, c1 = ubatch[k]
                if c1 <= c0:
                    return
                w = c1 - c0
                we = min(w + (w % 2), NPp - c0)  # keep matmul free dims even
                import contextlib
                with tc.high_priority() if k == 0 else contextlib.nullcontext():
                    for c in range(2):
                        # the slot masks are one-hot per piece column, so each
                        # G2 column receives exactly one nonzero term: building
                        # the fp16 G2 directly on the DVE is exact and skips
                        # the fp32 Gacc staging + ACT copy hop on every
                        # U-batch unlock chain
                        nc.vector.tensor_scalar_mul(
                            G2_sb[:, c, c0 : c0 + we], mask_sb[:, 0, c0 : c0 + we], g_sb[:, c, 0:1]
                        )
                        for s in range(1, k + 1):
                            tmp2 = tmpp.tile([128, NPp], F32, tag="tmp2", name=f"tmpe_{k}_{c}_{s}")
                            nc.vector.tensor_scalar_mul(
                                tmp2[:, :we], mask_sb[:, s, c0 : c0 + we], g_sb[:, c, s : s + 1]
                            )
                            nc.vector.tensor_add(
                                G2_sb[:, c, c0 : c0 + we], G2_sb[:, c, c0 : c0 + we], tmp2[:, :we]
                            )
                    for m in range(4):
                        psu = psA.tile([128, PT], F32, tag="psa", name=f"psu_{k}_{m}")
                        nc.tensor.matmul(
                            psu[:, :we], w3b_sb[:, 0, m, :], G2_sb[:, 0, c0 : c0 + we],
                            start=True, stop=False,
                        )
                        nc.tensor.matmul(
                            psu[:, :we], w3b_sb[:, 1, m, :], G2_sb[:, 1, c0 : c0 + we],
                            start=False, stop=True,
                        )
                        nc.scalar.activation(
                            out=U_sb[:, m, c0 : c0 + we], in_=psu[:, :we],
                            func=AF.Identity, bias=b3_sb[:, m : m + 1],
                        )

            h3_tiles = {}

            def emit_L3(t):
                L = lens[t]
                h3_sb = h3p.tile([128, 4, PT], F16, tag="h3", name=f"h3_{t}")
                for m in range(4):
                    ps3 = psB3.tile([128, PT], F32, tag="ps3", name=f"ps3_{t}_{m}")
                    nc.tensor.matmul(
                        ps3[:, :L], w3a_sb[:, 0, m, :], f2_all[:, t, 0, : L // QT, :], start=True, stop=False
                    )
                    nc.tensor.matmul(
                        ps3[:, :L], w3a_sb[:, 1, m, :], f2_all[:, t, 1, : L // QT, :], start=False, stop=True
                    )
                    for p in tile_pieces[t]:
                        _, a, b = pieces[p]
                        nc.scalar.activation(
                            out=h3_sb[:, m, a:b], in_=ps3[:, a:b], func=AF.Relu,
                            bias=U_sb[:, m, p : p + 1],
                        )
                h3_tiles[t] = h3_sb

            def emit_L4(t):
                L = lens[t]
                nq = L // QT
                ps = tile_pieces[t]
                h3_sb = h3_tiles.pop(t)
                for m in range(8):
                    # one PSUM bank (512 f32) per m-chunk
                    ps4 = psB4.tile([128, NQ, QT], F32, tag="ps4", name=f"ps4_{t}_{m}")
                    for k in range(4):
                        nc.tensor.matmul(
                            ps4[:, :nq, :], w4_sb[:, k, m, :], h3_sb[:, k, :L],
                            start=(k == 0), stop=(k == 3),
                        )
                    if len(ps) == 1:
                        nc.vector.tensor_reduce(
                            out=Mx4_sb[:, m : m + 1, ps[0] : ps[0] + 1],
                            in_=ps4[:, :nq, :], axis=mybir.AxisListType.XY, op=ALU_MAX,
                        )
                    else:
                        nc.vector.tensor_reduce(
                            out=Mx4_sb[:, m : m + 1, ps[0] : ps[0] + nq],
                            in_=ps4[:, :nq, :], axis=AXX, op=ALU_MAX,
                        )

            def emit_L4_pair(ta, tb):
                """L4 for the last two tiles with shared weight loads.

                The short final tile alone is LDWEIGHTS-bound (its ~85ns
                matmuls can't hide the ~95ns weight loads). Interleaving the
                two tiles' accumulation groups per (k, m) reuses each loaded
                weight for both, so every load hides behind a 216+85ns pair.
                """
                La, Lb = lens[ta], lens[tb]
                nqa, nqb = La // QT, Lb // QT
                h3a = h3_tiles.pop(ta)
                h3b = h3_tiles.pop(tb)
                for m in range(8):
                    p4a = psB4.tile([128, NQ, QT], F32, tag="ps4", name=f"ps4_{ta}_{m}")
                    p4b = psB4.tile([128, NQ, QT], F32, tag="ps4", name=f"ps4_{tb}_{m}")
                    for k in range(4):
                        nc.tensor.matmul(
                            p4a[:, :nqa, :], w4_sb[:, k, m, :], h3a[:, k, :La],
                            start=(k == 0), stop=(k == 3), skip_group_check=True,
                        )
                        nc.tensor.matmul(
                            p4b[:, :nqb, :], w4_sb[:, k, m, :], h3b[:, k, :Lb],
                            start=(k == 0), stop=(k == 3), skip_group_check=True,
                        )
                    for t, nq, p4 in ((ta, nqa, p4a), (tb, nqb, p4b)):
                        ps = tile_pieces[t]
                        if len(ps) == 1:
                            nc.vector.tensor_reduce(
                                out=Mx4_sb[:, m : m + 1, ps[0] : ps[0] + 1],
                                in_=p4[:, :nq, :], axis=mybir.AxisListType.XY, op=ALU_MAX,
                            )
                        else:
                            nc.vector.tensor_reduce(
                                out=Mx4_sb[:, m : m + 1, ps[0] : ps[0] + nq],
                                in_=p4[:, :nq, :], axis=AXX, op=ALU_MAX,
                            )

            # ---------------- interleaved pipeline ----------------
            a_next = 0
            b_next = 0
            l3_next = 0
            u_next = 0
            g_emitted = [False] * S
            mx4_sent = 0
            chunk1, chunk2 = T // 2, (4 * T) // 5

            def try_unlock():
                nonlocal u_next
                for s in range(S):
                    if not g_emitted[s] and a_next >= bounds[s]:
                        emit_g(s)
                        g_emitted[s] = True
                while u_next < S and all(g_emitted[s] for s in range(u_next + 1)):
                    emit_Ubatch(u_next)
                    u_next += 1

            def advance_A():
                nonlocal a_next
                ta = a_next
                tb = ta + 1 if ta + 1 < T else None
                emit_A_pair(ta, tb)
                a_next = ta + (1 if tb is None else 2)
                try_unlock()

            # phase A must lead phase B by enough tiles that B's U-table
            # chunks are always unlocked when its L3s reach the PE queue
            need = [bounds[ubatch_of_tile[min(i + 1, T - 1)]] for i in range(T)]
            LEAD = max(max(need[i] - i for i in range(T)) + 1, need[0])

            while b_next < T:
                while a_next < min(T, b_next + LEAD):
                    advance_A()
                progressed = False
                while (
                    l3_next <= min(b_next + 1, T - 1)
                    and ubatch_of_tile[l3_next] < u_next
                    and l3_next < a_next
                ):
                    emit_L3(l3_next)
                    l3_next += 1
                    progressed = True
                if l3_next > b_next:
                    if b_next == T - 2 and l3_next == T:
                        emit_L4_pair(T - 2, T - 1)
                        b_next = T
                    else:
                        emit_L4(b_next)
                        b_next += 1
                    progressed = True
                    # stream completed Mx4 columns out mid-kernel so the final
                    # DMA doesn't serialize behind the last tile
                    if b_next in (chunk1, chunk2):
                        pdone = max(tile_pieces[b_next - 1]) + 1
                        nc.gpsimd.dma_start(
                            out=mx4.ap()[:, :, mx4_sent:pdone], in_=Mx4_sb[:, :, mx4_sent:pdone]
                        )
                        mx4_sent = pdone
                if not progressed:
                    if a_next < T:
                        advance_A()
                    else:
                        raise RuntimeError("pipeline deadlock")

            nc.sync.dma_start(out=mx4.ap()[:, :, mx4_sent:], in_=Mx4_sb[:, :, mx4_sent:])

    nc.finalize()
    return nc


def _a128(v):
    return ((int(v) + QT - 1) // QT) * QT


def _prepare(x, seg_ids, B):
    """Dense-pack segments; boundaries QT-aligned inside shared window tiles."""
    counts = np.bincount(seg_ids, minlength=B)
    starts = np.concatenate([[0], np.cumsum(counts)])
    npts = counts.astype(int)
    S = (B + N_CORES - 1) // N_CORES

    # assign segments to cores (S per core), balancing total points
    order = np.argsort(-npts, kind="stable")
    assign: list[list[int]] = [[] for _ in range(N_CORES)]
    loads = [0] * N_CORES
    for s in order:
        cands = [c for c in range(N_CORES) if len(assign[c]) < S]
        c = min(cands, key=lambda i: loads[i])
        assign[c].append(int(s))
        loads[c] += int(npts[s])
    for _ in range(5000):
        hi = max(range(N_CORES), key=lambda i: loads[i])
        improved = False
        for lo in sorted(range(N_CORES), key=lambda i: loads[i]):
            if lo == hi:
                continue
            for ia, sa in enumerate(assign[hi]):
                for ib, sb in enumerate(assign[lo]):
                    d = npts[sa] - npts[sb]
                    if d > 0 and max(loads[hi] - d, loads[lo] + d) < max(loads[hi], loads[lo]):
                        assign[hi][ia], assign[lo][ib] = sb, sa
                        loads[hi] -= d
                        loads[lo] += d
                        improved = True
                        break
                if improved:
                    break
            if improved:
                break
        if not improved:
            break

    # per-core slot order: coordinate descent minimizing (T_cols, #window tiles)
    def plan(orders, LB=None):
        LB = LB or [0] * (S - 1)
        cur = np.zeros(N_CORES, dtype=int)
        wins = []
        for k in range(S - 1):
            nat = cur + np.array([npts[orders[c][k]] for c in range(N_CORES)])
            ali = np.maximum(np.array([_a128(v) for v in nat]), LB[k])
            wins.append((int(ali.min() // PT), int((ali.max() - 1) // PT)))
            cur = ali
        tot = cur + np.array([npts[orders[c][S - 1]] for c in range(N_CORES)])
        T_cols = _a128(int(tot.max()))
        nwin = sum(hi - lo + 1 for lo, hi in wins)
        return wins, T_cols, cur, tot, nwin

    orders = [sorted(assign[c], key=lambda s: -npts[s]) for c in range(N_CORES)]
    for _sweep in range(6):
        changed = False
        for c in range(N_CORES):
            bl, bo = None, None
            for perm in permutations(assign[c]):
                test = [list(o) for o in orders]
                test[c] = list(perm)
                _, T_cols, _, tot, nwin = plan(test)
                # tie-break: small slot-0 segments shorten the pipeline fill
                slot0 = max(npts[o[0]] for o in test)
                key = (T_cols, nwin, int(tot.max()), slot0)
                if bl is None or key < bl:
                    bl, bo = key, list(perm)
            if bo != orders[c]:
                changed = True
            orders[c] = bo
        if not changed:
            break

    # shrink windows from the left wherever lifting the low cores costs no
    # extra tiles (pads on non-critical cores are free)
    LB = [0] * (S - 1)
    base_T = plan(orders, LB)[1]
    for k in range(S - 2, -1, -1):
        while True:
            wins, T_cols, _, _, _ = plan(orders, LB)
            lo, hi = wins[k]
            if lo >= hi:
                break
            trial = list(LB)
            trial[k] = PT * (lo + 1)
            if plan(orders, trial)[1] <= base_T:
                LB = trial
            else:
                break
    wins, T_cols, _, _, _ = plan(orders, LB)
    wtiles = tuple(sorted(set(t for lo, hi in wins for t in range(lo, hi + 1))))

    T, lens, pieces, tile_pieces = _layout(T_cols, wtiles)
    NP = len(pieces)
    NPp = NP + (NP % 2)

    # per-core packed x and piece->slot masks
    xT_cores, mask_cores, maskA_cores, post = [], [], [], []
    for c in range(N_CORES):
        parts, cutcols = [], []
        cum = 0
        for s in range(S):
            seg = orders[c][s]
            pts = x[starts[seg] : starts[seg + 1]]
            width = (max(_a128(cum + len(pts)), LB[s]) if s < S - 1 else T_cols) - cum
            reps = (width + len(pts) - 1) // len(pts)
            if reps > 1:
                pts = np.concatenate([pts] * reps)
            parts.append(pts[:width])
            cum += width
            cutcols.append(cum)
        xc = np.concatenate(parts).astype(np.float16)
        xT_cores.append(np.ascontiguousarray(xc.T))
        # piece p (cols [ga, gb)) belongs to slot = #cuts <= ga
        pslot = np.empty(NP, dtype=int)
        for p, (t, a, b) in enumerate(pieces):
            ga = t * PT + a
            pslot[p] = sum(1 for cc in cutcols[:-1] if cc <= ga)
        m01 = np.zeros((S, NPp), np.float32)
        m01[pslot, np.arange(NP)] = 1.0
        mA = (m01 - 1.0) * np.float32(1e30)
        mask_cores.append(np.ascontiguousarray(np.broadcast_to(m01[None], (128, S, NPp))))
        maskA_cores.append(np.ascontiguousarray(np.broadcast_to(mA[None], (128, S, NPp))))
        post.append((orders[c], pslot))

    # shared pipeline bounds: slot s complete after its last piece's tile (max
    # over cores); with per-core pslot, use the mask to find the last piece.
    bounds = []
    for s in range(S):
        last_tile = 0
        for c in range(N_CORES):
            ps = np.flatnonzero(post[c][1] == s)
            if len(ps):
                last_tile = max(last_tile, pieces[ps.max()][0])
        bounds.append(last_tile + 1)
    smax = tuple(int(max(post[c][1][p] for c in range(N_CORES))) for p in range(NP))
    return T_cols, wtiles, S, tuple(bounds), smax, xT_cores, mask_cores, maskA_cores, post


def make_in_maps(inputs):
    """Fold BN, pack points, and build the per-core SPMD input dicts.

    Returns (key, in_maps, post, b4f) where key indexes _PROGRAM_CACHE.
    """
    x = np.asarray(inputs["x"], np.float32)
    seg_ids = np.asarray(inputs["seg_ids"])
    B = int(inputs["num_segments"])

    Wf, bf = [], []
    for i in (1, 2, 3, 4):
        W = np.asarray(inputs[f"W{i}"], np.float32)
        b = np.asarray(inputs[f"b{i}"], np.float32)
        ga = np.asarray(inputs[f"g{i}"], np.float32)
        be = np.asarray(inputs[f"be{i}"], np.float32)
        m = np.asarray(inputs[f"m{i}"], np.float32)
        v = np.asarray(inputs[f"v{i}"], np.float32)
        sc = ga / np.sqrt(v + EPS)
        Wf.append(np.ascontiguousarray(W * sc[None, :]))
        bf.append((b - m) * sc + be)
    W1f, W2f, W3f, W4f = Wf
    b1f, b2f, b3f, b4f = bf

    # f2 is stored shifted by -b2 on device; fold the constant back into b3.
    b3_eff = b3f + W3f[:256].T @ b2f + W3f[256:].T @ b2f
    # b2 == 0 (true for this reference's identity BN): the shift is a no-op,
    # which lets the c0 store run as a plain relu on ACT.
    zsh = bool(np.all(b2f == 0.0))

    (T_cols, wtiles, S, bounds, smax,
     xT_cores, mask_cores, maskA_cores, post) = _prepare(x, seg_ids, B)

    w1d = np.zeros((35, 128), np.float16)
    w1d[0:3] = W1f.astype(np.float16)
    w1d[32:35] = W1f.astype(np.float16)
    w2d = np.ascontiguousarray(W2f.reshape(128, 2, 128).astype(np.float16))
    w3ad = np.ascontiguousarray(W3f[:256].reshape(2, 128, 4, 128).transpose(1, 0, 2, 3).astype(np.float16))
    w3bd = np.ascontiguousarray(W3f[256:].reshape(2, 128, 4, 128).transpose(1, 0, 2, 3).astype(np.float16))
    w4d = np.ascontiguousarray(W4f.reshape(4, 128, 8, 128).transpose(1, 0, 2, 3).astype(np.float16))
    b1d = np.ascontiguousarray(b1f.reshape(128, 1))
    nb2d = np.ascontiguousarray(-b2f.reshape(2, 128).T)
    b3d = np.ascontiguousarray(b3_eff.reshape(4, 128).T)

    in_maps = [
        {
            "xT": xT_cores[c],
            "mask": mask_cores[c],
            "maskA": maskA_cores[c],
            "w1": w1d,
            "w2": w2d,
            "w3a": w3ad,
            "w3b": w3bd,
            "w4": w4d,
            "b1": b1d,
            "nb2": nb2d,
            "b3": b3d,
        }
        for c in range(N_CORES)
    ]
    return (T_cols, wtiles, S, bounds, smax, zsh), in_maps, post, b4f


def postprocess(results, post, b4f, B):
    out = np.zeros((B, 1024), np.float32)
    for c in range(N_CORES):
        mx4 = results[c]["mx4"]  # [128, 8, NP]
        segs, pslot = post[c]
        for s, seg in enumerate(segs):
            cols = np.flatnonzero(pslot == s)
            raw = mx4[:, :, cols].max(axis=2)  # [128, 8]
            out[seg] = np.maximum(raw.T.reshape(1024) + b4f, 0.0)
    return out


def get_program(key):
    if key not in _PROGRAM_CACHE:
        _PROGRAM_CACHE[key] = _build_program(*key)
    return _PROGRAM_CACHE[key]


def kernel(**inputs) -> np.ndarray:
    B = int(inputs["num_segments"])
    key, in_maps, post, b4f = make_in_maps(inputs)
    nc = get_program(key)
    last_err = None
    for _ in range(3):  # retry transient NRT device wedges
        try:
            res = run_bass_kernel_spmd(nc, in_maps, core_ids=list(range(N_CORES)))
            return postprocess(res.results, post, b4f, B)
        except Exception as e:  # noqa: BLE001
            last_err = e
    raise last_err
